# revision 30
# baseline (speedup 1.0000x reference)
"""Trainium2 Bass kernel for a 3-layer GCN+RGCN GNN (IGMC-style).

Contract: kernel(**inputs) takes FULL unsharded inputs (same keys as
setup_inputs()) and returns the FULL [100000, 64] float32 output.

Strategy (8 NeuronCores, SPMD):
  - Nodes sharded by destination: core k owns nodes [k*12500, (k+1)*12500),
    padded to 12544 = 98 tiles x 128 partitions.
  - Per message-passing round every core holds a replicated fp16 node table
    in local DRAM (rows padded to 256B), rebuilt by an AllGather of
    per-core shards.
  - Edge messages are fetched with the GPSIMD dma_gather custom op (one
    256B row per edge slot).  int16 indices limit a gather to 32K rows, so
    the table is split into 4 segments and edges are grouped by
    (dst-tile[, relation], src-segment), each group padded to 128-edge
    chunks (pad: index 0 + dst -1, neutralized by the indicator matmul).
  - Segment-sum runs on the tensor engine: per 128-edge chunk a fp16 0/1
    indicator S[e, m] = (dst_local[e] == m) is matmul-accumulated into
    PSUM.  S matrices are built in bulk (one DVE op per (batch, segment)).
  - GCN rounds gather a pre-transformed y = h @ Wn table (matmul commutes
    with segment-sum).  RGCN rounds gather raw h, accumulate per-relation
    transposed aggregates, then apply the 5 relation weights as small fp32
    matmuls.
  - Host-side preprocessing only does index manipulation / layout packing;
    all model FLOPs run on device.
"""

import sys
import math
import numpy as np

sys.path.insert(0, "/opt/trn_rl_repo")

N_CORES = 8
SEG_MAX = 32768          # int16 index reach per dma_gather


class Cfg:
    def __init__(self, n_nodes=100000, n_edges=1600000, n_rel=5,
                 d_in=128, d_h=64, d_out=64, group_g=8, group_r=4,
                 single_packet=False, batched_s=True):
        self.n_nodes = n_nodes
        self.n_edges = n_edges
        self.R = n_rel
        self.d_in = d_in
        self.d_h = d_h
        self.d_out = d_out
        self.own = n_nodes // N_CORES
        self.T = math.ceil(self.own / 128)
        self.pad = self.T * 128
        self.rows = N_CORES * 128 * self.T          # table rows
        self.n_seg = max(1, math.ceil(self.rows / SEG_MAX))
        # table piece p = source tiles [p*piece_T, ...) gathered separately
        self.piece_T = math.ceil(self.T / self.n_seg)
        self.piece_Ts = [min(self.piece_T, self.T - p * self.piece_T)
                         for p in range(self.n_seg)]
        self.piece_rows = [N_CORES * 128 * t for t in self.piece_Ts]
        assert max(self.piece_rows) <= SEG_MAX
        self.group_g = group_g
        self.group_r = group_r
        self.single_packet = single_packet
        self.batched_s = batched_s
        assert n_nodes % N_CORES == 0


# ---------------------------------------------------- host preprocessing ---

def _piece_rows(v, cfg):
    """Original node id -> (piece, row id inside piece-p gathered table).

    Piece p holds source tiles [p*piece_T, ...); its AG output viewed as
    [N_CORES*128*T_p, 128] has row = (core*128 + part)*T_p + (t - t0_p)
    for local node l = t*128 + part."""
    k = v // cfg.own
    l = v - k * cfg.own
    t, p = l // 128, l % 128
    piece = np.minimum(t // cfg.piece_T, cfg.n_seg - 1)
    t0 = piece * cfg.piece_T
    T_p = np.asarray(cfg.piece_Ts)[piece]
    return piece, (k * 128 + p) * T_p + (t - t0)


def pack_round(src, dst, rel, R, cfg, B):
    """Group edges by (dst tile, rel, src segment); build per-core int16
    gather indices, fp16 dst-local arrays, and the batch/chunk metadata
    shared by all cores (chunk counts are maxed over cores)."""
    NS = cfg.n_seg
    T = cfg.T
    core = dst // cfg.own
    l = dst - core * cfg.own
    tile = l // 128
    dloc = (l % 128).astype(np.float32)
    seg, lrow = _piece_rows(src, cfg)           # piece id, int16-safe row
    gr = rel if R > 1 else np.zeros_like(tile)

    # group id: (core, tile, rel, seg)
    gid = (((core * T + tile) * R + gr) * NS + seg).astype(np.int64)
    n_groups = N_CORES * T * R * NS
    order = np.argsort(gid, kind="stable")
    gid_s = gid[order]
    lrow_s = lrow[order]
    dloc_s = dloc[order]
    counts = np.bincount(gid_s, minlength=n_groups).reshape(N_CORES, T, R, NS)
    starts = np.concatenate([[0], np.cumsum(counts.ravel())])

    share = True            # share chunk streams across (tile, rel) runs
    K = np.ceil(counts.max(axis=0) / 128).astype(np.int64)   # [T, R, NS]

    # ---- batch/column layout (shared across cores) ----
    # consumers (dv/S columns) are separate from gather chunks; in the
    # non-shared layout they coincide 1:1.  All columns batch-relative.
    batches = []
    idx_c = 0
    chunk_c = 0
    dv_c = 0
    NB = math.ceil(T / B)
    for b in range(NB):
        ts = list(range(b * B, min((b + 1) * B, T)))
        segs = []
        tile_chunks = {t: [] for t in ts}
        cons = {}          # (t, s) -> (dv col, ca, cb)  [share mode]
        gbase = {}         # (t, r, s) -> chunk col      [padded mode]
        c0, d0, i0 = chunk_c, dv_c, idx_c
        for s in range(NS):
            nch = 0
            ncons = 0
            if share:
                # runs = (tile, rel) pairs in stream order
                # (index in two steps: numpy moves broadcast advanced dims
                # to the front when they are separated by a slice)
                cnt_k = counts[:, ts][..., s].reshape(N_CORES, len(ts) * R)
                nch = int(np.ceil(cnt_k.sum(axis=1).max() / 128))
                if nch == 0:
                    continue
                ends = np.cumsum(cnt_k, axis=1)
                begs = ends - cnt_k
                sc0 = chunk_c - c0                           # stream base
                for j in range(len(ts) * R):
                    t, r = ts[j // R], j % R
                    has = cnt_k[:, j] > 0
                    if not has.any():
                        continue
                    ca = int((begs[has, j] // 128).min())
                    cb = int(((ends[has, j] - 1) // 128).max())
                    cons[(t, r, s)] = (dv_c - d0, ca, cb)
                    for c in range(ca, cb + 1):
                        tile_chunks[t].append(
                            (dv_c - d0 + (c - ca), sc0 + c, r))
                    dv_c += cb + 1 - ca
                    ncons += cb + 1 - ca
                chunk_c += nch
            else:
                for t in ts:
                    for r in range(R):
                        k = int(K[t, r, s])
                        if k == 0:
                            continue
                        gbase[(t, r, s)] = chunk_c
                        for j in range(k):
                            tile_chunks[t].append(
                                (dv_c + j - d0, chunk_c + j - c0, r))
                        chunk_c += k
                        dv_c += k
                        nch += k
                        ncons += k
            if nch:
                segs.append(dict(s=s, n_chunks=nch, idx_c0=idx_c - i0,
                                 chunk_c0=(chunk_c - nch) - c0,
                                 n_cons=ncons, cons_c0=(dv_c - ncons) - d0))
                idx_c += nch * 8
        batches.append(dict(tiles=ts, idx_c0=i0, idx_cols=idx_c - i0,
                            chunk_c0=c0, chunks=chunk_c - c0,
                            dv_c0=d0, dvs=dv_c - d0, segs=segs,
                            tile_chunks=tile_chunks, cons=cons,
                            gbase=gbase))
    IC, CC, CD = idx_c, chunk_c, dv_c

    # ---- per-core slot filling ----
    metas = []
    for k in range(N_CORES):
        idxf = np.zeros((IC // 8) * 128, np.int16)     # flat idx stream
        dv = np.full((128, CD), -1.0, np.float32)
        for binfo in batches:
            iseg = {sd["s"]: sd for sd in binfo["segs"]}
            ts = binfo["tiles"]
            if share:
                for sd in binfo["segs"]:
                    s = sd["s"]
                    cnt_k = counts[k][ts][:, :, s].reshape(len(ts) * R)
                    begs = np.cumsum(cnt_k) - cnt_k
                    base = (binfo["idx_c0"] + sd["idx_c0"]) * 16
                    for j in range(len(ts) * R):
                        t, r = ts[j // R], j % R
                        n = int(cnt_k[j])
                        if n == 0:
                            continue
                        g = ((k * T + t) * R + r) * NS + s
                        s0, e0 = starts[g], starts[g + 1]
                        assert e0 - s0 == n
                        pos = begs[j] + np.arange(n)
                        idxf[base + pos] = lrow_s[s0:e0]
                        dq, ca, _cb = binfo["cons"][(t, r, s)]
                        dv[pos % 128, binfo["dv_c0"] + dq
                           + (pos // 128 - ca)] = dloc_s[s0:e0]
            else:
                for (t, r, s), cb in binfo["gbase"].items():
                    g = ((k * T + t) * R + r) * NS + s
                    s0, e0 = starts[g], starts[g + 1]
                    n = e0 - s0
                    if n == 0:
                        continue
                    pos = np.arange(n)
                    p = pos % 128
                    ch = cb + pos // 128
                    dv[p, binfo["dv_c0"] + (ch - binfo["chunk_c0"])] = \
                        dloc_s[s0:e0]
                    sd = iseg[s]
                    base = (binfo["idx_c0"] + sd["idx_c0"]) * 16
                    loc = (cb - (binfo["chunk_c0"] + sd["chunk_c0"])) * 128
                    idxf[base + loc + pos] = lrow_s[s0:e0]
        # wrap: idx i -> [i%16, i//16], replicated over 8 partition groups
        idx16 = np.tile(idxf.reshape(-1, 16).T, (8, 1))
        dv16 = dv.astype(np.float16).view(np.int16)
        metas.append(np.ascontiguousarray(
            np.concatenate([idx16, dv16], axis=1)))
    meta = dict(batches=batches, IC=IC, CC=CC, CD=CD)
    return metas, meta


def preprocess(inputs, cfg):
    x = np.asarray(inputs["x"], np.float32)
    ei = np.asarray(inputs["edge_index"], np.int64)
    rei = np.asarray(inputs["rel_edge_index"], np.int64)
    ret = np.asarray(inputs["rel_edge_type"], np.int64)

    g_m16, g_meta = pack_round(ei[0], ei[1], None, 1, cfg, cfg.group_g)
    r_m16, r_meta = pack_round(rei[0], rei[1], ret, cfg.R, cfg, cfg.group_r)

    xTs = []
    for k in range(N_CORES):
        xo = np.zeros((cfg.pad, cfg.d_in), np.float32)
        xo[:cfg.own] = x[k * cfg.own:(k + 1) * cfg.own]
        xTs.append(np.ascontiguousarray(xo.T))

    D = cfg.d_h
    cols = {}
    pieces = []
    c = 0

    def put(name, w):
        nonlocal c
        w = np.asarray(w, np.float32)
        pad = np.zeros((cfg.d_in, w.shape[1]), np.float32)
        pad[:w.shape[0]] = w
        cols[name] = (c, w.shape[1], w.shape[0])
        pieces.append(pad)
        c += w.shape[1]

    for ll in range(3):
        put(f"gWs{ll}", inputs[f"gWs{ll}"])
        put(f"gWn{ll}", inputs[f"gWn{ll}"])
        put(f"rWr{ll}", inputs[f"rWr{ll}"])
        for r in range(cfg.R):
            put(f"rW{ll}_{r}", np.asarray(inputs[f"rW{ll}"], np.float32)[r])
    put("Wout", inputs["Wout"])
    w_pack = np.concatenate(pieces, axis=1)

    bnames = ["gb0", "rb0", "gb1", "rb1", "gb2", "rb2", "bout"]
    bias_rows = [np.asarray(inputs[n], np.float32) for n in bnames]
    bias_pack = np.tile(np.concatenate(bias_rows)[None, :], (128, 1))
    bcols = {n: i * D for i, n in enumerate(bnames)}

    iota = np.tile(np.arange(128, dtype=np.float32)[None, :],
                   (128, 1)).astype(np.float16)
    ident = np.eye(128, dtype=np.float32)

    in_maps = []
    for k in range(N_CORES):
        in_maps.append({
            "xT": xTs[k],
            "g_meta": g_m16[k], "r_meta": r_m16[k],
            "w_pack": w_pack, "bias_pack": bias_pack,
            "iota": iota, "ident": ident,
        })
    meta = dict(g=g_meta, r=r_meta, wcols=cols, bcols=bcols)
    return in_maps, meta


# ------------------------------------------------------------- bass build ---

def build(cfg, meta, debug=False):
    import concourse.bass as bass
    import concourse.bacc as bacc
    import concourse.mybir as mybir
    import concourse.tile as tile

    dt = mybir.dt
    D = cfg.d_h
    T = cfg.T
    MDT = dt.float16
    ROWW = 128
    g_meta, r_meta = meta["g"], meta["r"]
    wcols, bcols = meta["wcols"], meta["bcols"]

    nc = bacc.Bacc(None, target_bir_lowering=False, num_devices=N_CORES)

    xT_in = nc.dram_tensor("xT", [cfg.d_in, T * 128], dt.float32,
                           kind="ExternalInput")
    g_meta_in = nc.dram_tensor("g_meta", [128, g_meta["IC"] + g_meta["CD"]],
                               dt.int16, kind="ExternalInput")
    r_meta_in = nc.dram_tensor("r_meta", [128, r_meta["IC"] + r_meta["CD"]],
                               dt.int16, kind="ExternalInput")
    n_wcols = sum(v[1] for v in wcols.values())
    w_in = nc.dram_tensor("w_pack", [cfg.d_in, n_wcols], dt.float32,
                          kind="ExternalInput")
    b_in = nc.dram_tensor("bias_pack", [128, 7 * D], dt.float32,
                          kind="ExternalInput")
    iota_in = nc.dram_tensor("iota", [128, 128], MDT,
                             kind="ExternalInput")
    ident_in = nc.dram_tensor("ident", [128, 128], dt.float32,
                              kind="ExternalInput")
    out_ext = nc.dram_tensor("out", [128, T * cfg.d_out], dt.float32,
                             kind="ExternalOutput")

    hTg = nc.dram_tensor("hTg", [D, T * 128], dt.float32)
    hTr = nc.dram_tensor("hTr", [D, T * 128], dt.float32)
    agin, agout = {}, {}
    for name in ["y0", "h0", "y1", "h1", "y2", "h2"]:
        agin[name] = [nc.dram_tensor(f"agin_{name}_{p}",
                                     [128, cfg.piece_Ts[p] * ROWW], MDT)
                      for p in range(cfg.n_seg)]
        agout[name] = [nc.dram_tensor(f"agout_{name}_{p}",
                                      [cfg.piece_rows[p], ROWW], MDT,
                                      addr_space="Shared")
                       for p in range(cfg.n_seg)]
    rg = [list(range(N_CORES))]

    g_IC = g_meta["IC"]
    r_IC = r_meta["IC"]
    g_mic = max(b["idx_cols"] for b in g_meta["batches"])
    g_mcc = max(b["chunks"] for b in g_meta["batches"])
    r_mic = max(b["idx_cols"] for b in r_meta["batches"])
    r_mcc = max(b["chunks"] for b in r_meta["batches"])
    g_mcd = max(b["dvs"] for b in g_meta["batches"])
    r_mcd = max(b["dvs"] for b in r_meta["batches"])
    mcc = max(g_mcc, r_mcc)
    mcd = max(g_mcd, r_mcd)

    import contextlib
    with tile.TileContext(nc) as tc, contextlib.ExitStack() as ctx:
        cpool = ctx.enter_context(tc.tile_pool(name="consts", bufs=1))
        lpool = ctx.enter_context(tc.tile_pool(name="loads", bufs=2))
        mpool = ctx.enter_context(tc.tile_pool(name="meta", bufs=4))
        gpool = ctx.enter_context(tc.tile_pool(name="gath", bufs=2))
        spool = ctx.enter_context(tc.tile_pool(name="smat", bufs=1))
        s1pool = ctx.enter_context(tc.tile_pool(name="smat1", bufs=6))
        hpool = ctx.enter_context(tc.tile_pool(name="work", bufs=3))
        stpool = ctx.enter_context(tc.tile_pool(name="stage", bufs=2))
        ppool = ctx.enter_context(tc.tile_pool(name="psum", bufs=2,
                                               space="PSUM"))

        w_sb = cpool.tile([cfg.d_in, n_wcols], dt.float32, tag="wsb")
        nc.sync.dma_start(out=w_sb[:], in_=w_in[:])
        b_sb = cpool.tile([128, 7 * D], dt.float32, tag="bsb")
        nc.sync.dma_start(out=b_sb[:], in_=b_in[:])
        iota_sb = cpool.tile([128, 128], MDT, tag="iosb")
        nc.sync.dma_start(out=iota_sb[:], in_=iota_in[:])
        id_sb = cpool.tile([128, 128], dt.float32, tag="idsb")
        nc.sync.dma_start(out=id_sb[:], in_=ident_in[:])

        def W(name):
            c0, n, kdim = wcols[name]
            return w_sb[:kdim, c0:c0 + n]

        def B(name):
            c0 = bcols[name]
            return b_sb[:, c0:c0 + D]

        def emit_gather(out_ap3, in_ap, idxs_ap, num_idxs):
            eng = nc.gpsimd
            _in = eng.lower_ap_dma(in_ap, for_custom_bir_dma=True)
            _idx = eng.lower_ap(idxs_ap)
            _out = eng.lower_ap(out_ap3)
            stride_bytes = ROWW * mybir.dt.size(in_ap.dtype)
            return eng.add_instruction(mybir.InstDMAGatherAnt(
                name=nc.get_next_instruction_name(),
                ins=[*_in, _idx, eng.lower_val_access(eng.to_reg(num_idxs))],
                outs=[_out],
                transpose=False,
                num_idxs=num_idxs,
                elem_size=ROWW,
                stride_bytes_256=stride_bytes // 256,
                gen_mode=0,
                single_packet=cfg.single_packet,
                queue_num=0,
                sbuf_tokens_per_rank=0,
                sbuf_free_dim_per_rank=0,
                sbuf_free_dim_pad_per_rank=0,
                sbuf_byte_offset=0))

        def do_gathers(binfo, idx_tile, tables, tag):
            """Issue per-piece dma_gathers for one batch; returns the
            gathered tile [128, chunks*ROWW] fp16."""
            nch = binfo["chunks"]
            gath = gpool.tile([128, mcc * ROWW], MDT, tag="g_x")
            for sd in binfo["segs"]:
                s, n = sd["s"], sd["n_chunks"]
                c0 = sd["chunk_c0"]
                emit_gather(
                    gath[:, c0 * ROWW:(c0 + n) * ROWW].rearrange(
                        "p (c d) -> p c d", d=ROWW),
                    tables[s][0:cfg.piece_rows[s], 0:ROWW],
                    idx_tile[:, sd["idx_c0"]:sd["idx_c0"] + n * 8],
                    n * 128)
            return gath

        ag_done = {}

        def write_agin(name, stg, t0, n):
            """Write stg tile rows [t0, t0+n) into the per-piece AG inputs."""
            for p in range(cfg.n_seg):
                p0 = p * cfg.piece_T
                p1 = p0 + cfg.piece_Ts[p]
                a, b = max(t0, p0), min(t0 + n, p1)
                if a >= b:
                    continue
                nc.sync.dma_start(
                    out=agin[name][p][:].rearrange(
                        "p (t w) -> p t w", w=ROWW)[:, a - p0:b - p0, 0:D],
                    in_=stg[:, (a - t0) * D:(b - t0) * D].rearrange(
                        "p (t d) -> p t d", d=D))

        def maybe_ag(name, tiles_done):
            """AllGather every piece fully staged once `tiles_done` tiles
            of the producing phase are complete."""
            done = ag_done.setdefault(name, set())
            for p in range(cfg.n_seg):
                if p in done:
                    continue
                if p * cfg.piece_T + cfg.piece_Ts[p] <= tiles_done:
                    nc.gpsimd.collective_compute(
                        "AllGather", mybir.AluOpType.bypass,
                        replica_groups=rg,
                        ins=[agin[name][p][:]], outs=[agout[name][p][:]])
                    done.add(p)

        def build_S(binfo, dst_tile):
            """One DVE op per (batch, segment): S for all chunks of the
            segment.  Returns tile [128, chunks*128] fp16 aligned with the
            batch-relative chunk columns."""
            nch = binfo["chunks"]
            S = spool.tile([128, mcd * 128], MDT, tag="Sb")
            for sd in binfo["segs"]:
                n = sd["n_cons"]
                c0 = sd["cons_c0"]
                nc.vector.tensor_tensor(
                    out=S[:, c0 * 128:(c0 + n) * 128].rearrange(
                        "p (c f) -> p c f", f=128),
                    in0=dst_tile[:, c0:c0 + n].rearrange(
                        "p (c one) -> p c one", one=1).to_broadcast(
                        [128, n, 128]),
                    in1=iota_sb[:].rearrange(
                        "p (one f) -> p one f", one=1).to_broadcast(
                        [128, n, 128]),
                    op=mybir.AluOpType.is_equal)
            return S

        def make_S(dst_tile, col):
            S = s1pool.tile([128, 128], MDT, tag="S")
            nc.vector.tensor_tensor(
                out=S[:],
                in0=dst_tile[:, col:col + 1].to_broadcast([128, 128]),
                in1=iota_sb[:],
                op=mybir.AluOpType.is_equal)
            return S

        def finish_h(psum_o, bias_ap):
            h = hpool.tile([128, D], dt.float32, tag="h")
            nc.vector.tensor_tensor(out=h[:], in0=psum_o[:], in1=bias_ap,
                                    op=mybir.AluOpType.add)
            nc.vector.tensor_relu(out=h[:], in_=h[:])
            return h

        def transpose_h(h):
            pt = ppool.tile([D, 128], dt.float32, tag="pt", space="PSUM")
            nc.tensor.transpose(out=pt[:], in_=h[:], identity=id_sb[:])
            hT = hpool.tile([D, 128], dt.float32, tag="hT")
            nc.scalar.copy(out=hT[:], in_=pt[:])
            return hT

        # ============ prologue: y0 = x @ gWn0 ============
        for binfo in g_meta["batches"]:
            ts = binfo["tiles"]
            t0, n = ts[0], len(ts)
            xt = lpool.tile([cfg.d_in, cfg.group_g * 128], dt.float32,
                            tag="xt")
            nc.sync.dma_start(out=xt[:, :n * 128],
                              in_=xT_in[:, t0 * 128:(t0 + n) * 128])
            stg = stpool.tile([128, cfg.group_g * D], MDT, tag="stg_y0")
            for i in range(n):
                py = ppool.tile([128, D], dt.float32, tag="py", space="PSUM")
                nc.tensor.matmul(out=py[:], lhsT=xt[:, i * 128:(i + 1) * 128],
                                 rhs=W("gWn0"), start=True, stop=True)
                nc.scalar.copy(out=stg[:, i * D:(i + 1) * D],
                               in_=py[:])
            write_agin("y0", stg, t0, n)
            maybe_ag("y0", t0 + n)

        # ============ rounds ============
        for ll in range(3):
            # ---- GCN ----
            for binfo in g_meta["batches"]:
                ts = binfo["tiles"]
                t0, n = ts[0], len(ts)
                nch = binfo["chunks"]
                ndv = binfo["dvs"]
                meta_t = mpool.tile([128, g_mic + g_mcd], dt.int16,
                                    tag="gmeta")
                nc.sync.dma_start(
                    out=meta_t[:, :binfo["idx_cols"]],
                    in_=g_meta_in[:, binfo["idx_c0"]:
                                  binfo["idx_c0"] + binfo["idx_cols"]])
                nc.sync.dma_start(
                    out=meta_t[:, g_mic:g_mic + ndv],
                    in_=g_meta_in[:, g_IC + binfo["dv_c0"]:
                                  g_IC + binfo["dv_c0"] + ndv])
                idx_t = meta_t[:, :binfo["idx_cols"]]
                dst_t = meta_t[:, g_mic:g_mic + ndv].bitcast(MDT)
                if ll == 0:
                    sT = lpool.tile([cfg.d_in, cfg.group_g * 128],
                                    dt.float32, tag="sTg0")
                    nc.sync.dma_start(out=sT[:, :n * 128],
                                      in_=xT_in[:, t0 * 128:(t0 + n) * 128])
                    sdim = cfg.d_in
                else:
                    sT = lpool.tile([D, cfg.group_g * 128], dt.float32,
                                    tag="sTg")
                    nc.sync.dma_start(out=sT[:, :n * 128],
                                      in_=hTr[:, t0 * 128:(t0 + n) * 128])
                    sdim = D
                if cfg.batched_s:
                    Sb = build_S(binfo, dst_t)
                gath = do_gathers(binfo, idx_t, agout[f"y{ll}"], "g")
                stg = stpool.tile([128, cfg.group_g * D], MDT, tag="stg_h")
                stgT = stpool.tile([D, cfg.group_g * 128], dt.float32,
                                   tag="stgT")
                for i, t in enumerate(ts):
                    po = ppool.tile([128, D], dt.float32, tag="po",
                                    space="PSUM")
                    chunks = binfo["tile_chunks"][t]
                    nc.tensor.matmul(out=po[:],
                                     lhsT=sT[:sdim, i * 128:(i + 1) * 128],
                                     rhs=W(f"gWs{ll}"), start=True,
                                     stop=(len(chunks) == 0),
                                     skip_group_check=True)
                    for j, (dcol, ccol, _r) in enumerate(chunks):
                        if cfg.batched_s:
                            S_ap = Sb[:, dcol * 128:(dcol + 1) * 128]
                        else:
                            S_ap = make_S(dst_t, dcol)[:]
                        nc.tensor.matmul(
                            out=po[:], lhsT=S_ap,
                            rhs=gath[:, ccol * ROWW:ccol * ROWW + D],
                            start=False, stop=(j == len(chunks) - 1),
                            skip_group_check=True)
                    h = finish_h(po, B(f"gb{ll}"))
                    nc.scalar.copy(out=stg[:, i * D:(i + 1) * D],
                                   in_=h[:])
                    hT = transpose_h(h)
                    nc.scalar.copy(out=stgT[:, i * 128:(i + 1) * 128],
                                   in_=hT[:])
                write_agin(f"h{ll}", stg, t0, n)
                maybe_ag(f"h{ll}", t0 + n)
                nc.sync.dma_start(out=hTg[:, t0 * 128:(t0 + n) * 128],
                                  in_=stgT[:, :n * 128])

            # ---- RGCN ----
            for binfo in r_meta["batches"]:
                ts = binfo["tiles"]
                t0, n = ts[0], len(ts)
                nch = binfo["chunks"]
                ndv = binfo["dvs"]
                meta_t = mpool.tile([128, r_mic + r_mcd], dt.int16,
                                    tag="rmeta")
                nc.sync.dma_start(
                    out=meta_t[:, :binfo["idx_cols"]],
                    in_=r_meta_in[:, binfo["idx_c0"]:
                                  binfo["idx_c0"] + binfo["idx_cols"]])
                nc.sync.dma_start(
                    out=meta_t[:, r_mic:r_mic + ndv],
                    in_=r_meta_in[:, r_IC + binfo["dv_c0"]:
                                  r_IC + binfo["dv_c0"] + ndv])
                idx_t = meta_t[:, :binfo["idx_cols"]]
                dst_t = meta_t[:, r_mic:r_mic + ndv].bitcast(MDT)
                sT = lpool.tile([D, cfg.group_r * 128], dt.float32,
                                tag="sTr")
                nc.sync.dma_start(out=sT[:, :n * 128],
                                  in_=hTg[:, t0 * 128:(t0 + n) * 128])
                if cfg.batched_s:
                    Sb = build_S(binfo, dst_t)
                gath = do_gathers(binfo, idx_t, agout[f"h{ll}"], "r")
                stgT = stpool.tile([D, cfg.group_r * 128], dt.float32,
                                   tag="stgTr")
                stg = stpool.tile([128, cfg.group_r * D], MDT, tag="stg_y2")
                stg_out = stpool.tile([128, cfg.group_r * D], dt.float32,
                                      tag="stg_out")
                for i, t in enumerate(ts):
                    po = ppool.tile([128, D], dt.float32, tag="po",
                                    space="PSUM")
                    nc.tensor.matmul(out=po[:],
                                     lhsT=sT[:, i * 128:(i + 1) * 128],
                                     rhs=W(f"rWr{ll}"), start=True,
                                     stop=False, skip_group_check=True)
                    by_rel = {}
                    for dcol, ccol, r in binfo["tile_chunks"][t]:
                        by_rel.setdefault(r, []).append((dcol, ccol))
                    rels = sorted(by_rel)
                    for ri, r in enumerate(rels):
                        cols = by_rel[r]
                        pa = ppool.tile([D, 128], dt.float32, tag="pa",
                                        space="PSUM")
                        for j, (dcol, ccol) in enumerate(cols):
                            if cfg.batched_s:
                                S_ap = Sb[:, dcol * 128:(dcol + 1) * 128]
                            else:
                                S_ap = make_S(dst_t, dcol)[:]
                            nc.tensor.matmul(
                                out=pa[:],
                                lhsT=gath[:, ccol * ROWW:ccol * ROWW + D],
                                rhs=S_ap,
                                start=(j == 0), stop=(j == len(cols) - 1),
                                skip_group_check=True)
                        aggT = hpool.tile([D, 128], dt.float32, tag="aggT")
                        nc.scalar.copy(out=aggT[:], in_=pa[:])
                        nc.tensor.matmul(out=po[:], lhsT=aggT[:],
                                         rhs=W(f"rW{ll}_{r}"), start=False,
                                         stop=(ri == len(rels) - 1),
                                         skip_group_check=True)
                    h = finish_h(po, B(f"rb{ll}"))
                    hT = transpose_h(h)
                    nc.scalar.copy(out=stgT[:, i * 128:(i + 1) * 128],
                                   in_=hT[:])
                    py = ppool.tile([128, D], dt.float32, tag="py",
                                    space="PSUM")
                    if ll < 2:
                        nc.tensor.matmul(out=py[:], lhsT=hT[:],
                                         rhs=W(f"gWn{ll + 1}"),
                                         start=True, stop=True)
                        nc.scalar.copy(out=stg[:, i * D:(i + 1) * D],
                                       in_=py[:])
                    else:
                        nc.tensor.matmul(out=py[:], lhsT=hT[:],
                                         rhs=W("Wout"), start=True, stop=True)
                        ot = hpool.tile([128, D], dt.float32, tag="ot")
                        nc.vector.tensor_tensor(out=ot[:], in0=py[:],
                                                in1=B("bout"),
                                                op=mybir.AluOpType.add)
                        nc.vector.tensor_relu(
                            out=stg_out[:, i * D:(i + 1) * D], in_=ot[:])
                nc.sync.dma_start(out=hTr[:, t0 * 128:(t0 + n) * 128],
                                  in_=stgT[:, :n * 128])
                if ll < 2:
                    write_agin(f"y{ll + 1}", stg, t0, n)
                    maybe_ag(f"y{ll + 1}", t0 + n)
                else:
                    nc.sync.dma_start(out=out_ext[:, t0 * D:(t0 + n) * D],
                                      in_=stg_out[:, :n * D])

    nc.compile()
    return nc


# ---------------------------------------------------------------- driver ---

_CACHE = {}


def _run(inputs, cfg, debug=False, trace=False, trace_cores=None):
    from concourse.bass_utils import run_bass_kernel_spmd

    in_maps, meta = preprocess(inputs, cfg)
    key = (cfg.n_nodes, cfg.n_edges, cfg.group_g, cfg.group_r,
           cfg.single_packet, cfg.batched_s, debug,
           meta["g"]["IC"], meta["r"]["IC"])
    if key not in _CACHE:
        _CACHE[key] = build(cfg, meta, debug=debug)
    nc = _CACHE[key]
    kw = {}
    if trace:
        kw = dict(trace=True, trace_cores=trace_cores or [0])
    res = run_bass_kernel_spmd(nc, in_maps, core_ids=list(range(N_CORES)),
                               **kw)
    outs = []
    for k in range(N_CORES):
        o = np.asarray(res.results[k]["out"])
        o = o.reshape(128, cfg.T, cfg.d_out).transpose(1, 0, 2)
        outs.append(o.reshape(cfg.pad, cfg.d_out)[:cfg.own])
    full = np.concatenate(outs, axis=0)
    return full, res


def kernel(**inputs):
    cfg = Cfg()
    full, _ = _run(inputs, cfg)
    return full.astype(np.float32)
